# revision 22
# baseline (speedup 1.0000x reference)
"""GAT+LSTM fused kernel for 8 trn2 NeuronCores (v3).

- Output depends only on batch row T-1=11 of the reference LSTM (ys[:, -1]),
  so only GAT outputs for nodes [110000, 120000) ("live" nodes) are needed.
- Edges sharded by src-range across 8 cores (only edges with a live dst);
  self-loops injected as ordinary edges (edge_attr slot filled on-device with
  mean(edge_attr), computed locally from the full edge_attr + gpsimd
  partition_all_reduce - no collective on the critical path).
- Per-edge x rows fetched with batched dma_gather (transpose=True -> the
  gathered tile arrives feature-major, bf16, <=896 idxs per instruction);
  h+a_src computed per edge slot with one bf16 matmul per 128-slot chunk.
- a_dst per slot via a tiny PE matmul: host-sent transposed one-hot
  (node -> slot) times an SBUF-resident per-bucket a_dst table (no gather).
- Segment softmax without max-subtraction; exp via sigmoid(q)/sigmoid(-q)
  (Exp's ACT table is not resident).
- Aggregation: host groups each core's edges by destination bucket
  (dst % 80 -> contiguous node ownership after ReduceScatter), exactly 2
  chunks of 128 per bucket; host sends the one-hot (slot -> node//80)
  matrices; a bf16 PE matmul accumulates messages per bucket into PSUM.
- Partial accumulators combined with a bf16 ReduceScatter; normalize + ReLU
  on 128 partitions; each core keeps its contiguous 1280-node gat slice
  locally (node-major in DRAM) and AllGathers only a 160-node halo.
- LSTM: each core solves its 1280-node slice + 160-node halo by fixed-point
  iteration (ITERS passes, tensor_tensor_scan for the cell recurrence).
  Core 0's halo is neutralized by forcing i/f gates to -30 (c,h -> 0).
- FC per core on its slice; small AllGather assembles the full output.
"""
import os
import numpy as np
import ml_dtypes

import concourse.bass as bass
import concourse.bass_isa as bass_isa
import concourse.bacc as bacc
import concourse.tile as tile
from concourse import mybir, library_config
from concourse.bass_utils import run_bass_kernel_spmd
from contextlib import ExitStack

dt = mybir.dt
F32 = dt.float32
BF16 = dt.bfloat16
I16 = dt.int16
AF = mybir.ActivationFunctionType
ALU = mybir.AluOpType
BF = ml_dtypes.bfloat16

T, N, F_IN = 12, 10000, 64
HEADS, C, HID = 4, 32, 32
E, TN = 1_000_000, 120_000
NCORES = 8
NSH = TN // NCORES              # 15000 src nodes per shard
D0 = (T - 1) * N                # 110000: first live node
DN = N
DBLK = 80                       # live-node buckets; bucket = n % 80
DNP = DBLK * 128                # 10240 padded live nodes
DSL = DNP // NCORES             # 1280 live nodes owned per core (contiguous)
BSLOT = 256                     # canonical slots per bucket (2 chunks)
NCH = DBLK * 2                  # 160 chunks
CAP = NCH * 128                 # 20480 slots
NPASS = 4
PC = NCH // NPASS               # 40 chunks per pass
PCAP = PC * 128                 # 5120 slots per pass
RX = 16384                      # x_rows table rows (shard + self + pad)
XPAD = NSH + DSL                # 16280: zero row
EAW = 7816                      # full-ea tile cols (128*7816 >= 1e6)
HALO = 160
SEQ = HALO + DSL                # 1440 LSTM columns per core
ITERS = 4
LEAK = 0.2

_CACHE = {}


def _bf(a):
    return np.asarray(a, np.float32).astype(BF)


def _wrap16(idx, cap):
    out = np.zeros((16, cap // 16), np.int16)
    j = np.arange(len(idx))
    out[j % 16, j // 16] = np.asarray(idx).astype(np.int16)
    return np.tile(out, (8, 1))


def _chunkify(vals, cap, fill):
    out = np.full(cap, fill, np.float32)
    out[:len(vals)] = vals
    return np.ascontiguousarray(out.reshape(cap // 128, 128).T)


def _prep_host(inputs):
    x = np.ascontiguousarray(np.asarray(inputs["x_seq"], np.float32).reshape(TN, F_IN))
    ei = np.asarray(inputs["edge_index"])
    ea = np.asarray(inputs["edge_attr"], np.float32)[:, 0]
    W_gat = np.asarray(inputs["W_gat"], np.float32)
    att_src = np.asarray(inputs["att_src"], np.float32)
    att_dst = np.asarray(inputs["att_dst"], np.float32)
    att_edge = np.asarray(inputs["att_edge"], np.float32)
    W_edge = np.asarray(inputs["W_edge"], np.float32)
    gat_bias = np.asarray(inputs["gat_bias"], np.float32)
    W_ih = np.asarray(inputs["W_ih"], np.float32)
    W_hh = np.asarray(inputs["W_hh"], np.float32)
    b = np.asarray(inputs["b_ih"], np.float32) + np.asarray(inputs["b_hh"], np.float32)
    W_fc = np.asarray(inputs["W_fc"], np.float32)
    b_fc = np.asarray(inputs["b_fc"], np.float32)

    A_src = np.zeros((HEADS * C, HEADS), np.float32)
    A_dst = np.zeros((HEADS * C, HEADS), np.float32)
    for h in range(HEADS):
        A_src[h * C:(h + 1) * C, h] = att_src[h]
        A_dst[h * C:(h + 1) * C, h] = att_dst[h]
    wbig = _bf(np.concatenate([W_gat, W_gat @ A_src], axis=1))       # [64,132]
    wad = _bf(W_gat @ A_dst)                                          # [64,4]
    kap = np.array([np.dot(W_edge[0, h * C:(h + 1) * C], att_edge[h])
                    for h in range(HEADS)], np.float32)
    kap_rep = np.broadcast_to(kap, (128, HEADS)).astype(np.float32).copy()
    gb_rep = _bf(np.broadcast_to(gat_bias, (128, HEADS * C)))
    perm = np.concatenate([np.arange(32, 64), np.arange(0, 32),
                           np.arange(96, 128), np.arange(64, 96)])
    WihT = _bf(np.ascontiguousarray(W_ih[perm].T))                   # [128,128]
    WhhT = np.ascontiguousarray(W_hh[perm].T)                        # [32,128] f32
    br = np.ascontiguousarray(b[perm].reshape(128, 1))

    src, dst = ei[0].astype(np.int64), ei[1].astype(np.int64)
    live = (dst >= D0) & (dst < D0 + DN)
    core_of = src // NSH
    xl = np.zeros((DNP, F_IN), np.float32)
    xl[:DN] = x[D0:D0 + DN]
    # bucket-major live-x for the a_dst table: tile b = nodes {p*80+b}
    no = (np.arange(128)[None, :] * DBLK + np.arange(DBLK)[:, None]).ravel()
    xTD_bf = _bf(xl[no].T)                                           # [64, DNP]
    eaFull = np.zeros((128, EAW), BF)
    j3 = np.arange(E)
    eaFull[j3 % 128, j3 // 128] = _bf(ea)
    in_maps = []
    for k in range(NCORES):
        m = live & (core_of == k)
        sL = src[m] - k * NSH
        dL = dst[m] - D0
        eav = ea[m]
        own = np.arange(k * DSL, (k + 1) * DSL)
        # x_rows table: shard rows, then own live rows, then zeros
        x_rows = np.zeros((RX, 128), BF)
        x_rows[:NSH, :F_IN] = _bf(x[k * NSH:(k + 1) * NSH])
        ov = own[own < DN]
        x_rows[NSH:NSH + len(ov), :F_IN] = _bf(x[D0 + ov])
        bflat = dL % DBLK
        hs_idx = np.full(CAP, XPAD, np.int64)
        eac = np.zeros(CAP, np.float32)
        ohp = np.full(CAP, -1, np.int64)       # slot -> dst//80, -1 = none
        selfm = np.zeros(CAP, np.float32)
        for bkt in range(DBLK):
            sel = np.nonzero(bflat == bkt)[0]
            sn = own[own % DBLK == bkt]        # 16 self nodes in this bucket
            nb = len(sel) + len(sn)
            assert nb <= BSLOT, f"core {k} bucket {bkt}: {nb} > {BSLOT}"
            o = bkt * BSLOT
            ne = len(sel)
            hs_idx[o:o + ne] = sL[sel]
            ohp[o:o + ne] = dL[sel] // DBLK
            eac[o:o + ne] = eav[sel]
            hs_idx[o + ne:o + nb] = NSH + (sn - k * DSL)
            ohp[o + ne:o + nb] = sn // DBLK
            selfm[o + ne:o + nb] = 1.0
        s_all = np.nonzero(ohp >= 0)[0]
        ohC = np.zeros((128, NCH * 128), BF)
        ohC[s_all % 128, (s_all // 128) * 128 + ohp[s_all]] = BF(1.0)
        ohT = np.zeros((128, NCH * 128), BF)
        ohT[ohp[s_all], (s_all // 128) * 128 + s_all % 128] = BF(1.0)
        # halo gather rows: 160 from left neighbor + 96 dups (unused)
        hrow = np.zeros(256, np.int64)
        hrow[:HALO] = ((k - 1) % NCORES) * HALO + np.arange(HALO)
        # core-0 halo kill pattern (perm order: f,i,o,g)
        hm = np.full((128, 1), 0.0 if k == 0 else 1.0, np.float32)
        hk = np.zeros((128, 1), np.float32)
        if k == 0:
            hk[0:64] = -30.0
        in_maps.append({
            "x_rows": x_rows, "xTD": xTD_bf, "ohC": ohC, "ohT": ohT,
            "eaC": _chunkify(eac, CAP, 0.0),
            "selfM": _chunkify(selfm, CAP, 0.0),
            "eaFull": eaFull,
            "hsI16": _wrap16(hs_idx, CAP),
            "slH16": _wrap16(hrow, 256),
            "wbig": wbig, "wad": wad, "kap": kap_rep, "gbias": gb_rep,
            "wih": WihT, "whh": WhhT, "br": br,
            "hm": hm, "hk": hk,
            "wfc": np.ascontiguousarray(W_fc.reshape(HID, 1)),
            "bfc": np.ascontiguousarray(b_fc.reshape(1, 1)),
        })
    return in_maps


def _build_nc(debug=False):
    STAGE = int(os.environ.get("KSTAGE", "99"))
    nc = bacc.Bacc("TRN2", target_bir_lowering=False, debug=False,
                   num_devices=NCORES)
    g = lambda n, s, d=F32: nc.dram_tensor(n, s, d, kind="ExternalInput").ap()
    x_rows = g("x_rows", [RX, 128], BF16)
    xTD = g("xTD", [F_IN, DNP], BF16)
    ohC = g("ohC", [128, NCH * 128], BF16)
    ohT = g("ohT", [128, NCH * 128], BF16)
    eaC = g("eaC", [128, NCH]); selfM = g("selfM", [128, NCH])
    eaFull = g("eaFull", [128, EAW], BF16)
    hsI16 = g("hsI16", [128, CAP // 16], I16)
    slH16 = g("slH16", [128, 16], I16)
    wbig = g("wbig", [F_IN, 132], BF16); wad = g("wad", [F_IN, 4], BF16)
    kap = g("kap", [128, HEADS]); gbias = g("gbias", [128, 128], BF16)
    wih = g("wih", [128, 128], BF16); whh = g("whh", [HID, 128])
    br = g("br", [128, 1]); hm = g("hm", [128, 1]); hk = g("hk", [128, 1])
    wfc = g("wfc", [HID, 1]); bfc = g("bfc", [1, 1])
    out = nc.dram_tensor("out", [NCORES, DSL], F32, kind="ExternalOutput").ap()
    if debug:
        dbg_gt = nc.dram_tensor("dbg_gt", [128, SEQ], F32, kind="ExternalOutput").ap()
        dbg_gx = nc.dram_tensor("dbg_gx", [128, SEQ], F32, kind="ExternalOutput").ap()
        dbg_h = nc.dram_tensor("dbg_h", [HID, SEQ], F32, kind="ExternalOutput").ap()
        dbg_acct = nc.dram_tensor("dbg_acct", [128, DBLK * 132], F32,
                                  kind="ExternalOutput").ap()
        dbg_gbn = nc.dram_tensor("dbg_gbn", [128, 10 * 128], F32,
                                 kind="ExternalOutput").ap()

    acc_tbl = nc.dram_tensor("acc_tbl", [128, DBLK * 132], BF16).ap()
    rs_out = nc.dram_tensor("rs_out", [16, DBLK * 132], BF16).ap()
    gat_loc = nc.dram_tensor("gat_loc", [DSL, 128], BF16).ap()
    ghalo_in = nc.dram_tensor("ghalo_in", [HALO, 128], BF16).ap()
    ghalo_full = nc.dram_tensor("ghalo_full", [NCORES * HALO, 128], BF16,
                                addr_space="Shared").ap()
    fc_out = nc.dram_tensor("fc_out", [1, DSL], F32).ap()
    fc_full = nc.dram_tensor("fc_full", [NCORES, DSL], F32,
                             addr_space="Shared").ap()
    RG = [list(range(NCORES))]

    with tile.TileContext(nc) as tc, ExitStack() as top:
        const = top.enter_context(tc.tile_pool(name="const", bufs=1))
        nc.gpsimd.load_library(library_config.mlp)
        # small/meta loads first so the gathers can start early
        hsI_t = const.tile([128, CAP // 16], I16)
        nc.sync.dma_start(hsI_t[:], hsI16[:])
        wbig_t = const.tile([F_IN, 132], BF16); nc.sync.dma_start(wbig_t[:], wbig[:])
        wad_t = const.tile([F_IN, 4], BF16); nc.sync.dma_start(wad_t[:], wad[:])
        kap_t = const.tile([128, HEADS], F32); nc.sync.dma_start(kap_t[:], kap[:])
        ea_t = const.tile([128, NCH], F32); nc.sync.dma_start(ea_t[:], eaC[:])
        sm_t = const.tile([128, NCH], F32); nc.sync.dma_start(sm_t[:], selfM[:])
        gb_t = const.tile([128, 128], BF16); nc.sync.dma_start(gb_t[:], gbias[:])
        wih_t = const.tile([128, 128], BF16); nc.sync.dma_start(wih_t[:], wih[:])
        whh_t = const.tile([HID, 128], F32); nc.sync.dma_start(whh_t[:], whh[:])
        br_t = const.tile([128, 1], F32); nc.sync.dma_start(br_t[:], br[:])
        hm_t = const.tile([128, 1], F32); nc.sync.dma_start(hm_t[:], hm[:])
        hk_t = const.tile([128, 1], F32); nc.sync.dma_start(hk_t[:], hk[:])
        wfc_t = const.tile([HID, 1], F32); nc.sync.dma_start(wfc_t[:], wfc[:])
        bfc_t = const.tile([1, 1], F32); nc.sync.dma_start(bfc_t[:], bfc[:])
        slH_t = const.tile([128, 16], I16); nc.sync.dma_start(slH_t[:], slH16[:])
        meanr = const.tile([128, 1], F32)
        eap = const.tile([128, 1], F32)

        # a_dst table: ADTb[p, b*4:(b+1)*4] = a_dst of node p*80+b (bf16)
        ADTb = top.enter_context(tc.tile_pool(name="adt", bufs=1)) \
            .tile([128, DBLK * 4], BF16)
        if STAGE >= 1:
            with ExitStack() as ph:
                sba = ph.enter_context(tc.tile_pool(name="sba", bufs=1))
                psq = ph.enter_context(tc.tile_pool(name="psq", bufs=1, space="PSUM"))
                xtd_t = sba.tile([F_IN, DNP], BF16)
                nc.sync.dma_start(xtd_t[:], xTD[:])
                pack_d = psq.tile([128, DBLK * 4], F32, space="PSUM", tag="pkd")
                for j in range(DBLK):
                    nc.tensor.matmul(pack_d[:, j * 4:(j + 1) * 4],
                                     lhsT=xtd_t[:, j * 128:(j + 1) * 128],
                                     rhs=wad_t[:], start=True, stop=True)
                nc.vector.tensor_copy(ADTb[:], pack_d[:])

        # mean(edge_attr): chunked local reduce + PE partition broadcast
        sbm = top.enter_context(tc.tile_pool(name="sbm", bufs=2))
        with ExitStack() as ph:
            psm = ph.enter_context(tc.tile_pool(name="psm", bufs=1, space="PSUM"))
            EQ = EAW // 4
            eapp = sbm.tile([128, 4], F32, bufs=1)
            for q in range(4):
                eaf_t = sbm.tile([128, EQ], BF16, tag="eaf")
                nc.sync.dma_start(eaf_t[:], eaFull[:, q * EQ:(q + 1) * EQ])
                nc.vector.tensor_reduce(eapp[:, q:q + 1], eaf_t[:],
                                        mybir.AxisListType.X, ALU.add)
            nc.vector.tensor_reduce(eap[:], eapp[:], mybir.AxisListType.X, ALU.add)
            onc = sbm.tile([128, 1], F32)
            nc.vector.memset(onc[:], 1.0)
            onr = sbm.tile([1, 128], F32)
            nc.vector.memset(onr[:], 1.0)
            ps1 = psm.tile([1, 1], F32, space="PSUM", tag="ps1")
            nc.tensor.matmul(ps1[:], lhsT=eap[:], rhs=onc[:], start=True, stop=True)
            eas = sbm.tile([1, 1], F32)
            nc.scalar.mul(eas[:], ps1[:], 1.0 / E)
            ps2 = psm.tile([128, 1], F32, space="PSUM", tag="ps2")
            nc.tensor.matmul(ps2[:], lhsT=onr[:], rhs=eas[:], start=True, stop=True)
            nc.vector.tensor_copy(meanr[:], ps2[:])


        persist = top.enter_context(tc.tile_pool(name="persist", bufs=1))
        ACCT = persist.tile([128, DBLK * 132], BF16)

        if STAGE >= 2:
            # ---------- edge phase, NPASS passes of PC chunks ----------
            with ExitStack() as ph:
                sbp = ph.enter_context(tc.tile_pool(name="sbp", bufs=2))
                pse = ph.enter_context(tc.tile_pool(name="pse", bufs=2, space="PSUM"))
                ps2 = ph.enter_context(tc.tile_pool(name="ps2", bufs=2, space="PSUM"))
                psa = ph.enter_context(tc.tile_pool(name="psa", bufs=4, space="PSUM"))

                def _emit_agg(prv):
                    pc0, poh, pSCv = prv
                    for cc in range(PC):
                        cg = pc0 + cc
                        bkt = cg // 2
                        first = (cg % 2 == 0)
                        last = (cg % 2 == 1)
                        if first:
                            pac = psa.tile([128, 132], F32, space="PSUM", tag="pacc")
                            _emit_agg.pac = pac
                        pac = _emit_agg.pac
                        nc.tensor.matmul(pac[:], lhsT=poh[:, cc * 128:(cc + 1) * 128],
                                         rhs=pSCv[:, cc, :], start=first, stop=last)
                        if last:
                            nc.vector.tensor_copy(
                                ACCT[:, bkt * 132:(bkt + 1) * 132], pac[:])
                    blo, bhi = pc0 // 2, (pc0 + PC) // 2
                    nc.sync.dma_start(acc_tbl[:, blo * 132:bhi * 132],
                                      ACCT[:, blo * 132:bhi * 132])

                xg = sbp.tile([128, 1, CAP], BF16, tag="xg", bufs=1)
                # <=896 idxs per transpose-gather (DGE per-inst limit)
                off = 0
                while off < CAP:
                    n = min(896, CAP - off)
                    nc.gpsimd.dma_gather(
                        xg[:, :, off:off + n], x_rows[:],
                        hsI_t[:, off // 16:(off + n) // 16],
                        n, n, 128, transpose=True)
                    off += n
                xgv = xg[:].rearrange("p a b -> p (a b)")
                prev = None
                for p in range(NPASS):
                    c0 = p * PC
                    oh_t = sbp.tile([128, PC * 128], BF16, tag="oh")
                    nc.sync.dma_start(oh_t[:], ohC[:, c0 * 128:(c0 + PC) * 128])
                    ohT_t = sbp.tile([128, PC * 128], BF16, tag="ohT")
                    nc.sync.dma_start(ohT_t[:], ohT[:, c0 * 128:(c0 + PC) * 128])
                    SH = sbp.tile([128, PC * 132], BF16, tag="SH")
                    SHv = SH[:].rearrange("p (e w) -> p e w", w=132)
                    padst = ps2.tile([128, PC * 4], F32, space="PSUM", tag="padst")
                    for cc in range(PC):
                        if cc % 2 == 0:
                            ph_ = pse.tile([128, 264], F32, space="PSUM", tag="ph")
                        nc.tensor.matmul(ph_[:, (cc % 2) * 132:(cc % 2) * 132 + 132],
                                         lhsT=xgv[0:F_IN, (c0 + cc) * 128:(c0 + cc + 1) * 128],
                                         rhs=wbig_t[:], start=True, stop=True)
                        nc.tensor.matmul(padst[:, cc * 4:(cc + 1) * 4],
                                         lhsT=ohT_t[:, cc * 128:(cc + 1) * 128],
                                         rhs=ADTb[:, ((c0 + cc) // 2) * 4:
                                                  ((c0 + cc) // 2) * 4 + 4],
                                         start=True, stop=True)
                        if cc % 2 == 1:
                            nc.scalar.mul(SH[:, (cc - 1) * 132:(cc + 1) * 132],
                                          ph_[:], 1.0)
                    if prev is not None:
                        _emit_agg(prev)
                    # alpha math, batched over the pass
                    ADx = sbp.tile([128, PC * 4], BF16, tag="ADx", bufs=1)
                    nc.vector.tensor_copy(ADx[:], padst[:])
                    ADxv = ADx[:].rearrange("p (e w) -> p e w", w=4)
                    EAm = sbp.tile([128, PC], F32, tag="EAm", bufs=1)
                    nc.vector.scalar_tensor_tensor(
                        out=EAm[:], in0=sm_t[:, c0:c0 + PC], scalar=meanr[:],
                        op0=ALU.mult, op1=ALU.add, in1=ea_t[:, c0:c0 + PC])
                    Q = sbp.tile([128, PC * 4], F32, tag="Q", bufs=1)
                    Qv = Q[:].rearrange("p (e w) -> p e w", w=4)
                    nc.vector.tensor_tensor(out=Qv, in0=SHv[:, :, 128:132],
                                            in1=ADxv, op=ALU.add)
                    T2 = sbp.tile([128, PC * 4], F32, tag="T2", bufs=1)
                    T2v = T2[:].rearrange("p (e w) -> p e w", w=4)
                    ea3 = EAm[:].rearrange("p (e w) -> p e w", w=1) \
                        .to_broadcast([128, PC, 4])
                    kap3 = kap_t[:].rearrange("p (o w) -> p o w", o=1) \
                        .to_broadcast([128, PC, 4])
                    nc.vector.tensor_tensor(out=T2v, in0=ea3, in1=kap3, op=ALU.mult)
                    nc.vector.tensor_tensor(out=Qv, in0=Qv, in1=T2v, op=ALU.add)
                    nc.vector.tensor_scalar_mul(T2v, Qv, LEAK)
                    nc.vector.tensor_tensor(out=Qv, in0=Qv, in1=T2v, op=ALU.max)
                    SG1 = sbp.tile([128, PC * 4], F32, tag="SG1", bufs=1)
                    nc.scalar.activation(SG1[:], Q[:], AF.Sigmoid)
                    S4 = sbp.tile([128, PC * 4], F32, tag="S4", bufs=1)
                    nc.scalar.activation(S4[:], Q[:], AF.Sigmoid, scale=-1.0)
                    nc.vector.reciprocal(S4[:], S4[:])
                    nc.vector.tensor_tensor(out=S4[:], in0=SG1[:], in1=S4[:],
                                            op=ALU.mult)
                    S4b = sbp.tile([128, PC * 4], BF16, tag="S4b", bufs=1)
                    nc.vector.tensor_copy(S4b[:], S4[:])
                    S4bv = S4b[:].rearrange("p (e w) -> p e w", w=4)
                    SCT = sbp.tile([128, PC * 132], BF16, tag="SCT")
                    SCv = SCT[:].rearrange("p (e w) -> p e w", w=132)
                    nc.vector.tensor_copy(SCv[:, :, 128:132], S4bv)
                    nh4 = SHv[:, :, 0:128].rearrange("p e (h c) -> p e h c", h=HEADS)
                    sc4 = S4bv.rearrange("p e (h c) -> p e h c", c=1) \
                        .to_broadcast([128, PC, HEADS, C])
                    out4 = SCv[:, :, 0:128].rearrange("p e (h c) -> p e h c", h=HEADS)
                    nc.vector.tensor_tensor(out=out4, in0=nh4, in1=sc4, op=ALU.mult)
                    prev = (c0, oh_t, SCv)
                _emit_agg(prev)
            if debug:
                ACF = persist.tile([128, DBLK * 132], F32)
                nc.vector.tensor_copy(ACF[:], ACCT[:])
                nc.sync.dma_start(dbg_acct[:], ACF[:])

        if STAGE >= 3:
            # ---------- ReduceScatter (bf16) ----------
            nc.gpsimd.collective_compute("ReduceScatter", ALU.add, replica_groups=RG,
                                         ins=[acc_tbl[:]], outs=[rs_out[:]])

            # ---------- normalize + ReLU on 128 partitions ----------
            with ExitStack() as ph:
                sbn = ph.enter_context(tc.tile_pool(name="sbn", bufs=1))
                RSb = sbn.tile([128, 10 * 132], BF16)
                nc.sync.dma_start(RSb[:],
                                  rs_out[:].rearrange("p (a w) -> (p a) w", a=8))
                RSv = RSb[:].rearrange("p (j w) -> p j w", w=132)
                D4 = sbn.tile([128, 40], F32)
                D4v = D4[:].rearrange("p (j w) -> p j w", w=4)
                nc.vector.tensor_scalar_add(D4v, RSv[:, :, 128:132], 1e-16)
                nc.vector.reciprocal(D4[:], D4[:])
                RC = sbn.tile([128, 40], BF16)
                nc.vector.tensor_copy(RC[:], D4[:])
                RCv = RC[:].rearrange("p (j w) -> p j w", w=4)
                GBn = sbn.tile([128, 10 * 128], BF16)
                g4 = GBn[:].rearrange("p (j h c) -> p j h c", j=10, h=HEADS)
                m4 = RSv[:, :, 0:128].rearrange("p j (h c) -> p j h c", h=HEADS)
                r4 = RCv.rearrange("p j (h c) -> p j h c", c=1) \
                    .to_broadcast([128, 10, HEADS, C])
                nc.vector.tensor_tensor(out=g4, in0=m4, in1=r4, op=ALU.mult)
                GBv = GBn[:].rearrange("p (j w) -> p j w", w=128)
                gbb = gb_t[:].rearrange("p (o w) -> p o w", o=1) \
                    .to_broadcast([128, 10, 128])
                nc.vector.tensor_tensor(out=GBv, in0=GBv, in1=gbb, op=ALU.add)
                nc.vector.tensor_scalar_max(GBn[:], GBn[:], 0.0)
                # node-major writes: node t = r*80 + a*10 + bb  (partition = r*8+a)
                nc.sync.dma_start(
                    gat_loc[:].rearrange("(ra bb) w -> ra (bb w)", bb=10), GBn[:])
                nc.sync.dma_start(
                    ghalo_in[:].rearrange("(ra bb) w -> ra (bb w)", bb=10),
                    GBn[112:128, :])
                if debug:
                    GBF = sbn.tile([128, 10 * 128], F32)
                    nc.vector.tensor_copy(GBF[:], GBn[:])
                    nc.sync.dma_start(dbg_gbn[:], GBF[:])

        if STAGE >= 4:
            # ---------- halo AllGather + slice assembly ----------
            nc.gpsimd.collective_compute("AllGather", ALU.bypass, replica_groups=RG,
                                         ins=[ghalo_in[:]], outs=[ghalo_full[:]])

        lstm = top.enter_context(tc.tile_pool(name="lstm", bufs=1))
        GT = lstm.tile([128, SEQ], BF16)
        gx = lstm.tile([128, SEQ], F32)
        H = lstm.tile([HID, SEQ + 1], F32)
        nc.vector.memset(H[:, 0:1], 0.0)
        if STAGE < 5:
            nc.vector.memset(gx[:], 0.0)
            nc.vector.memset(GT[:], 0.0)
            nc.vector.memset(H[:], 0.0)
        if STAGE >= 5:
            with ExitStack() as ph:
                sbg = ph.enter_context(tc.tile_pool(name="sbg", bufs=1))
                psg = ph.enter_context(tc.tile_pool(name="psg", bufs=2, space="PSUM"))
                GTH = sbg.tile([128, 1, 256], BF16)
                nc.gpsimd.dma_gather(GTH[:], ghalo_full[:], slH_t[:],
                                     256, 256, 128, transpose=True)
                nc.vector.tensor_copy(GT[:, 0:HALO],
                                      GTH[:].rearrange("p a b -> p (a b)")[:, 0:HALO])
                nc.sync.dma_start_transpose(GT[:, HALO:SEQ], gat_loc[:])
                for lo, w in ((0, 512), (512, 512), (1024, 416)):
                    pg = psg.tile([128, 512], F32, space="PSUM", tag="pg")
                    nc.tensor.matmul(pg[:, 0:w], lhsT=wih_t[:], rhs=GT[:, lo:lo + w],
                                     start=True, stop=True)
                    nc.vector.tensor_scalar_add(gx[:, lo:lo + w], pg[:, 0:w], br_t[:])
                # halo kill (core 0): gx = gx*hm + hk
                nc.vector.scalar_tensor_tensor(
                    out=gx[:, 0:HALO], in0=gx[:, 0:HALO], scalar=hm_t[:],
                    op0=ALU.mult, op1=ALU.add,
                    in1=hk_t[:].to_broadcast([128, HALO]))
        if debug:
            dgt = lstm.tile([128, SEQ], F32)
            nc.vector.tensor_copy(dgt[:], GT[:])
            nc.sync.dma_start(dbg_gt[:], dgt[:])
            nc.sync.dma_start(dbg_gx[:], gx[:])

        if STAGE >= 6:
            # ---------- LSTM fixed point on [*, SEQ] ----------
            with ExitStack() as ph:
                sbl = ph.enter_context(tc.tile_pool(name="sbl", bufs=2))
                psl = ph.enter_context(tc.tile_pool(name="psl", bufs=1, space="PSUM"))
                use_f32r = os.environ.get("KF32R", "0") == "1"
                whh_r = whh_t[:].bitcast(dt.float32r) if use_f32r else whh_t[:]
                for it in range(ITERS):
                    if it == 0:
                        Gp = gx[:]
                    else:
                        pG = psl.tile([128, SEQ], F32, space="PSUM", tag="pG")
                        for lo, w in ((0, 512), (512, 512), (1024, 416)):
                            rh = H[:, lo:lo + w]
                            if use_f32r:
                                rh = rh.bitcast(dt.float32r)
                            nc.tensor.matmul(pG[:, lo:lo + w], lhsT=whh_r,
                                             rhs=rh, start=True, stop=True)
                        Gs = sbl.tile([128, SEQ], F32, tag="Gs")
                        nc.vector.tensor_tensor(out=Gs[:], in0=pG[:], in1=gx[:],
                                                op=ALU.add)
                        Gp = Gs[:]
                    S_ = sbl.tile([96, SEQ], F32, tag="S")
                    nc.scalar.activation(S_[:], Gp[0:96, :], AF.Sigmoid)
                    Tg = sbl.tile([64, SEQ], F32, tag="Tg")
                    nc.scalar.activation(Tg[32:64, :], Gp[96:128, :], AF.Tanh)
                    Zt = sbl.tile([HID, SEQ], F32, tag="Zt")
                    nc.vector.tensor_tensor(out=Zt[:], in0=S_[32:64, :],
                                            in1=Tg[32:64, :], op=ALU.mult)
                    Ct = sbl.tile([HID, SEQ], F32, tag="Ct")
                    nc.vector.tensor_tensor_scan(
                        out=Ct[:], data0=S_[0:32, :], data1=Zt[:],
                        initial=0.0, op0=ALU.mult, op1=ALU.add)
                    TC = sbl.tile([96, SEQ], F32, tag="TC")
                    nc.scalar.activation(TC[64:96, :], Ct[:], AF.Tanh)
                    nc.vector.tensor_tensor(out=H[:, 1:SEQ + 1], in0=S_[64:96, :],
                                            in1=TC[64:96, :], op=ALU.mult)
        if debug:
            nc.sync.dma_start(dbg_h[:], H[:, 1:SEQ + 1])

        if STAGE >= 7:
            # ---------- FC + final AllGather ----------
            with ExitStack() as ph:
                sbf = ph.enter_context(tc.tile_pool(name="sbf", bufs=1))
                psf = ph.enter_context(tc.tile_pool(name="psf", bufs=2, space="PSUM"))
                OF = sbf.tile([1, DSL], F32)
                for lo, w in ((0, 512), (512, 512), (1024, 256)):
                    pf = psf.tile([1, 512], F32, space="PSUM", tag="pf")
                    nc.tensor.matmul(pf[:, 0:w], lhsT=wfc_t[:],
                                     rhs=H[:, HALO + 1 + lo:HALO + 1 + lo + w],
                                     start=True, stop=True)
                    nc.vector.tensor_scalar_add(OF[:, lo:lo + w], pf[:, 0:w], bfc_t[:])
                nc.sync.dma_start(fc_out[:], OF[:])
                nc.gpsimd.collective_compute("AllGather", ALU.bypass,
                                             replica_groups=RG,
                                             ins=[fc_out[:]], outs=[fc_full[:]])
                OT = sbf.tile([NCORES, DSL], F32)
                nc.sync.dma_start(OT[:], fc_full[:])
                nc.sync.dma_start(out[:], OT[:])

    nc.compile()
    return nc


def run(inputs, trace=False, debug=False):
    key = ("dbg" if debug else "rel")
    if key not in _CACHE:
        _CACHE[key] = _build_nc(debug=debug)
    nc = _CACHE[key]
    in_maps = _prep_host(inputs)
    res = run_bass_kernel_spmd(nc, in_maps, list(range(NCORES)), trace=trace)
    return res


def kernel(**inputs) -> np.ndarray:
    res = run(inputs)
    o = res.results[0]["out"]
    return np.ascontiguousarray(
        o.reshape(NCORES * DSL)[:N].reshape(N, 1).astype(np.float32))


# revision 23
# speedup vs baseline: 1.0056x; 1.0056x over previous
"""GAT+LSTM fused kernel for 8 trn2 NeuronCores (v3).

- Output depends only on batch row T-1=11 of the reference LSTM (ys[:, -1]),
  so only GAT outputs for nodes [110000, 120000) ("live" nodes) are needed.
- Edges sharded by src-range across 8 cores (only edges with a live dst);
  self-loops injected as ordinary edges (edge_attr slot filled on-device with
  mean(edge_attr), computed locally from the full edge_attr + gpsimd
  partition_all_reduce - no collective on the critical path).
- Per-edge x rows fetched with batched dma_gather (transpose=True -> the
  gathered tile arrives feature-major, bf16, <=896 idxs per instruction);
  h+a_src computed per edge slot with one bf16 matmul per 128-slot chunk.
- a_dst per slot via a tiny PE matmul: host-sent transposed one-hot
  (node -> slot) times an SBUF-resident per-bucket a_dst table (no gather).
- Segment softmax without max-subtraction; exp via sigmoid(q)/sigmoid(-q)
  (Exp's ACT table is not resident).
- Aggregation: host groups each core's edges by destination bucket
  (dst % 80 -> contiguous node ownership after ReduceScatter), exactly 2
  chunks of 128 per bucket; host sends the one-hot (slot -> node//80)
  matrices; a bf16 PE matmul accumulates messages per bucket into PSUM.
- Partial accumulators combined with a bf16 ReduceScatter; normalize + ReLU
  on 128 partitions; each core keeps its contiguous 1280-node gat slice
  locally (node-major in DRAM) and AllGathers only a 160-node halo.
- LSTM: each core solves its 1280-node slice + 160-node halo by fixed-point
  iteration (ITERS passes, tensor_tensor_scan for the cell recurrence).
  Core 0's halo is neutralized by forcing i/f gates to -30 (c,h -> 0).
- FC per core on its slice; small AllGather assembles the full output.
"""
import os
import numpy as np
import ml_dtypes

import concourse.bass as bass
import concourse.bass_isa as bass_isa
import concourse.bacc as bacc
import concourse.tile as tile
from concourse import mybir, library_config
from concourse.bass_utils import run_bass_kernel_spmd
from contextlib import ExitStack

dt = mybir.dt
F32 = dt.float32
BF16 = dt.bfloat16
I16 = dt.int16
AF = mybir.ActivationFunctionType
ALU = mybir.AluOpType
BF = ml_dtypes.bfloat16

T, N, F_IN = 12, 10000, 64
HEADS, C, HID = 4, 32, 32
E, TN = 1_000_000, 120_000
NCORES = 8
NSH = TN // NCORES              # 15000 src nodes per shard
D0 = (T - 1) * N                # 110000: first live node
DN = N
DBLK = 80                       # live-node buckets; bucket = n % 80
DNP = DBLK * 128                # 10240 padded live nodes
DSL = DNP // NCORES             # 1280 live nodes owned per core (contiguous)
BSLOT = 256                     # canonical slots per bucket (2 chunks)
NCH = DBLK * 2                  # 160 chunks
CAP = NCH * 128                 # 20480 slots
NPASS = 8
PC = NCH // NPASS               # 40 chunks per pass
PCAP = PC * 128                 # 5120 slots per pass
RX = 16384                      # x_rows table rows (shard + self + pad)
XPAD = NSH + DSL                # 16280: zero row
EAW = 7816                      # full-ea tile cols (128*7816 >= 1e6)
HALO = 160
SEQ = HALO + DSL                # 1440 LSTM columns per core
ITERS = 4
LEAK = 0.2

_CACHE = {}


def _bf(a):
    return np.asarray(a, np.float32).astype(BF)


def _wrap16(idx, cap):
    out = np.zeros((16, cap // 16), np.int16)
    j = np.arange(len(idx))
    out[j % 16, j // 16] = np.asarray(idx).astype(np.int16)
    return np.tile(out, (8, 1))


def _chunkify(vals, cap, fill):
    out = np.full(cap, fill, np.float32)
    out[:len(vals)] = vals
    return np.ascontiguousarray(out.reshape(cap // 128, 128).T)


def _prep_host(inputs):
    x = np.ascontiguousarray(np.asarray(inputs["x_seq"], np.float32).reshape(TN, F_IN))
    ei = np.asarray(inputs["edge_index"])
    ea = np.asarray(inputs["edge_attr"], np.float32)[:, 0]
    W_gat = np.asarray(inputs["W_gat"], np.float32)
    att_src = np.asarray(inputs["att_src"], np.float32)
    att_dst = np.asarray(inputs["att_dst"], np.float32)
    att_edge = np.asarray(inputs["att_edge"], np.float32)
    W_edge = np.asarray(inputs["W_edge"], np.float32)
    gat_bias = np.asarray(inputs["gat_bias"], np.float32)
    W_ih = np.asarray(inputs["W_ih"], np.float32)
    W_hh = np.asarray(inputs["W_hh"], np.float32)
    b = np.asarray(inputs["b_ih"], np.float32) + np.asarray(inputs["b_hh"], np.float32)
    W_fc = np.asarray(inputs["W_fc"], np.float32)
    b_fc = np.asarray(inputs["b_fc"], np.float32)

    A_src = np.zeros((HEADS * C, HEADS), np.float32)
    A_dst = np.zeros((HEADS * C, HEADS), np.float32)
    for h in range(HEADS):
        A_src[h * C:(h + 1) * C, h] = att_src[h]
        A_dst[h * C:(h + 1) * C, h] = att_dst[h]
    wbig = _bf(np.concatenate([W_gat, W_gat @ A_src], axis=1))       # [64,132]
    wad = _bf(W_gat @ A_dst)                                          # [64,4]
    kap = np.array([np.dot(W_edge[0, h * C:(h + 1) * C], att_edge[h])
                    for h in range(HEADS)], np.float32)
    kap_rep = np.broadcast_to(kap, (128, HEADS)).astype(np.float32).copy()
    gb_rep = _bf(np.broadcast_to(gat_bias, (128, HEADS * C)))
    perm = np.concatenate([np.arange(32, 64), np.arange(0, 32),
                           np.arange(96, 128), np.arange(64, 96)])
    WihT = _bf(np.ascontiguousarray(W_ih[perm].T))                   # [128,128]
    WhhT = np.ascontiguousarray(W_hh[perm].T)                        # [32,128] f32
    br = np.ascontiguousarray(b[perm].reshape(128, 1))

    src, dst = ei[0].astype(np.int64), ei[1].astype(np.int64)
    live = (dst >= D0) & (dst < D0 + DN)
    core_of = src // NSH
    xl = np.zeros((DNP, F_IN), np.float32)
    xl[:DN] = x[D0:D0 + DN]
    # bucket-major live-x for the a_dst table: tile b = nodes {p*80+b}
    no = (np.arange(128)[None, :] * DBLK + np.arange(DBLK)[:, None]).ravel()
    xTD_bf = _bf(xl[no].T)                                           # [64, DNP]
    eaFull = np.zeros((128, EAW), BF)
    j3 = np.arange(E)
    eaFull[j3 % 128, j3 // 128] = _bf(ea)
    in_maps = []
    for k in range(NCORES):
        m = live & (core_of == k)
        sL = src[m] - k * NSH
        dL = dst[m] - D0
        eav = ea[m]
        own = np.arange(k * DSL, (k + 1) * DSL)
        # x_rows table: shard rows, then own live rows, then zeros
        x_rows = np.zeros((RX, 128), BF)
        x_rows[:NSH, :F_IN] = _bf(x[k * NSH:(k + 1) * NSH])
        ov = own[own < DN]
        x_rows[NSH:NSH + len(ov), :F_IN] = _bf(x[D0 + ov])
        bflat = dL % DBLK
        hs_idx = np.full(CAP, XPAD, np.int64)
        eac = np.zeros(CAP, np.float32)
        ohp = np.full(CAP, -1, np.int64)       # slot -> dst//80, -1 = none
        selfm = np.zeros(CAP, np.float32)
        for bkt in range(DBLK):
            sel = np.nonzero(bflat == bkt)[0]
            sn = own[own % DBLK == bkt]        # 16 self nodes in this bucket
            nb = len(sel) + len(sn)
            assert nb <= BSLOT, f"core {k} bucket {bkt}: {nb} > {BSLOT}"
            o = bkt * BSLOT
            ne = len(sel)
            hs_idx[o:o + ne] = sL[sel]
            ohp[o:o + ne] = dL[sel] // DBLK
            eac[o:o + ne] = eav[sel]
            hs_idx[o + ne:o + nb] = NSH + (sn - k * DSL)
            ohp[o + ne:o + nb] = sn // DBLK
            selfm[o + ne:o + nb] = 1.0
        s_all = np.nonzero(ohp >= 0)[0]
        ohC = np.zeros((128, NCH * 128), BF)
        ohC[s_all % 128, (s_all // 128) * 128 + ohp[s_all]] = BF(1.0)
        ohT = np.zeros((128, NCH * 128), BF)
        ohT[ohp[s_all], (s_all // 128) * 128 + s_all % 128] = BF(1.0)
        # halo gather rows: 160 from left neighbor + 96 dups (unused)
        hrow = np.zeros(256, np.int64)
        hrow[:HALO] = ((k - 1) % NCORES) * HALO + np.arange(HALO)
        # core-0 halo kill pattern (perm order: f,i,o,g)
        hm = np.full((128, 1), 0.0 if k == 0 else 1.0, np.float32)
        hk = np.zeros((128, 1), np.float32)
        if k == 0:
            hk[0:64] = -30.0
        in_maps.append({
            "x_rows": x_rows, "xTD": xTD_bf, "ohC": ohC, "ohT": ohT,
            "eaC": _chunkify(eac, CAP, 0.0),
            "selfM": _chunkify(selfm, CAP, 0.0),
            "eaFull": eaFull,
            "hsI16": _wrap16(hs_idx, CAP),
            "slH16": _wrap16(hrow, 256),
            "wbig": wbig, "wad": wad, "kap": kap_rep, "gbias": gb_rep,
            "wih": WihT, "whh": WhhT, "br": br,
            "hm": hm, "hk": hk,
            "wfc": np.ascontiguousarray(W_fc.reshape(HID, 1)),
            "bfc": np.ascontiguousarray(b_fc.reshape(1, 1)),
        })
    return in_maps


def _build_nc(debug=False):
    STAGE = int(os.environ.get("KSTAGE", "99"))
    nc = bacc.Bacc("TRN2", target_bir_lowering=False, debug=False,
                   num_devices=NCORES)
    g = lambda n, s, d=F32: nc.dram_tensor(n, s, d, kind="ExternalInput").ap()
    x_rows = g("x_rows", [RX, 128], BF16)
    xTD = g("xTD", [F_IN, DNP], BF16)
    ohC = g("ohC", [128, NCH * 128], BF16)
    ohT = g("ohT", [128, NCH * 128], BF16)
    eaC = g("eaC", [128, NCH]); selfM = g("selfM", [128, NCH])
    eaFull = g("eaFull", [128, EAW], BF16)
    hsI16 = g("hsI16", [128, CAP // 16], I16)
    slH16 = g("slH16", [128, 16], I16)
    wbig = g("wbig", [F_IN, 132], BF16); wad = g("wad", [F_IN, 4], BF16)
    kap = g("kap", [128, HEADS]); gbias = g("gbias", [128, 128], BF16)
    wih = g("wih", [128, 128], BF16); whh = g("whh", [HID, 128])
    br = g("br", [128, 1]); hm = g("hm", [128, 1]); hk = g("hk", [128, 1])
    wfc = g("wfc", [HID, 1]); bfc = g("bfc", [1, 1])
    out = nc.dram_tensor("out", [NCORES, DSL], F32, kind="ExternalOutput").ap()
    if debug:
        dbg_gt = nc.dram_tensor("dbg_gt", [128, SEQ], F32, kind="ExternalOutput").ap()
        dbg_gx = nc.dram_tensor("dbg_gx", [128, SEQ], F32, kind="ExternalOutput").ap()
        dbg_h = nc.dram_tensor("dbg_h", [HID, SEQ], F32, kind="ExternalOutput").ap()
        dbg_acct = nc.dram_tensor("dbg_acct", [128, DBLK * 132], F32,
                                  kind="ExternalOutput").ap()
        dbg_gbn = nc.dram_tensor("dbg_gbn", [128, 10 * 128], F32,
                                 kind="ExternalOutput").ap()

    acc_tbl = nc.dram_tensor("acc_tbl", [128, DBLK * 132], BF16).ap()
    rs_out = nc.dram_tensor("rs_out", [16, DBLK * 132], BF16).ap()
    gat_loc = nc.dram_tensor("gat_loc", [DSL, 128], BF16).ap()
    ghalo_in = nc.dram_tensor("ghalo_in", [HALO, 128], BF16).ap()
    ghalo_full = nc.dram_tensor("ghalo_full", [NCORES * HALO, 128], BF16,
                                addr_space="Shared").ap()
    fc_out = nc.dram_tensor("fc_out", [1, DSL], F32).ap()
    fc_full = nc.dram_tensor("fc_full", [NCORES, DSL], F32,
                             addr_space="Shared").ap()
    RG = [list(range(NCORES))]

    with tile.TileContext(nc) as tc, ExitStack() as top:
        const = top.enter_context(tc.tile_pool(name="const", bufs=1))
        nc.gpsimd.load_library(library_config.mlp)
        # small/meta loads first so the gathers can start early
        hsI_t = const.tile([128, CAP // 16], I16)
        nc.sync.dma_start(hsI_t[:], hsI16[:])
        wbig_t = const.tile([F_IN, 132], BF16); nc.sync.dma_start(wbig_t[:], wbig[:])
        wad_t = const.tile([F_IN, 4], BF16); nc.sync.dma_start(wad_t[:], wad[:])
        kap_t = const.tile([128, HEADS], F32); nc.sync.dma_start(kap_t[:], kap[:])
        ea_t = const.tile([128, NCH], F32); nc.sync.dma_start(ea_t[:], eaC[:])
        sm_t = const.tile([128, NCH], F32); nc.sync.dma_start(sm_t[:], selfM[:])
        gb_t = const.tile([128, 128], BF16); nc.sync.dma_start(gb_t[:], gbias[:])
        wih_t = const.tile([128, 128], BF16); nc.sync.dma_start(wih_t[:], wih[:])
        whh_t = const.tile([HID, 128], F32); nc.sync.dma_start(whh_t[:], whh[:])
        br_t = const.tile([128, 1], F32); nc.sync.dma_start(br_t[:], br[:])
        hm_t = const.tile([128, 1], F32); nc.sync.dma_start(hm_t[:], hm[:])
        hk_t = const.tile([128, 1], F32); nc.sync.dma_start(hk_t[:], hk[:])
        wfc_t = const.tile([HID, 1], F32); nc.sync.dma_start(wfc_t[:], wfc[:])
        bfc_t = const.tile([1, 1], F32); nc.sync.dma_start(bfc_t[:], bfc[:])
        slH_t = const.tile([128, 16], I16); nc.sync.dma_start(slH_t[:], slH16[:])
        meanr = const.tile([128, 1], F32)
        eap = const.tile([128, 1], F32)

        # a_dst table: ADTb[p, b*4:(b+1)*4] = a_dst of node p*80+b (bf16)
        ADTb = top.enter_context(tc.tile_pool(name="adt", bufs=1)) \
            .tile([128, DBLK * 4], BF16)
        if STAGE >= 1:
            with ExitStack() as ph:
                sba = ph.enter_context(tc.tile_pool(name="sba", bufs=1))
                psq = ph.enter_context(tc.tile_pool(name="psq", bufs=1, space="PSUM"))
                xtd_t = sba.tile([F_IN, DNP], BF16)
                nc.sync.dma_start(xtd_t[:], xTD[:])
                pack_d = psq.tile([128, DBLK * 4], F32, space="PSUM", tag="pkd")
                for j in range(DBLK):
                    nc.tensor.matmul(pack_d[:, j * 4:(j + 1) * 4],
                                     lhsT=xtd_t[:, j * 128:(j + 1) * 128],
                                     rhs=wad_t[:], start=True, stop=True)
                nc.vector.tensor_copy(ADTb[:], pack_d[:])

        # mean(edge_attr): chunked local reduce + PE partition broadcast
        sbm = top.enter_context(tc.tile_pool(name="sbm", bufs=2))
        with ExitStack() as ph:
            psm = ph.enter_context(tc.tile_pool(name="psm", bufs=1, space="PSUM"))
            EQ = EAW // 4
            eapp = sbm.tile([128, 4], F32, bufs=1)
            for q in range(4):
                eaf_t = sbm.tile([128, EQ], BF16, tag="eaf")
                nc.sync.dma_start(eaf_t[:], eaFull[:, q * EQ:(q + 1) * EQ])
                nc.vector.tensor_reduce(eapp[:, q:q + 1], eaf_t[:],
                                        mybir.AxisListType.X, ALU.add)
            nc.vector.tensor_reduce(eap[:], eapp[:], mybir.AxisListType.X, ALU.add)
            onc = sbm.tile([128, 1], F32)
            nc.vector.memset(onc[:], 1.0)
            onr = sbm.tile([1, 128], F32)
            nc.vector.memset(onr[:], 1.0)
            ps1 = psm.tile([1, 1], F32, space="PSUM", tag="ps1")
            nc.tensor.matmul(ps1[:], lhsT=eap[:], rhs=onc[:], start=True, stop=True)
            eas = sbm.tile([1, 1], F32)
            nc.scalar.mul(eas[:], ps1[:], 1.0 / E)
            ps2 = psm.tile([128, 1], F32, space="PSUM", tag="ps2")
            nc.tensor.matmul(ps2[:], lhsT=onr[:], rhs=eas[:], start=True, stop=True)
            nc.vector.tensor_copy(meanr[:], ps2[:])


        persist = top.enter_context(tc.tile_pool(name="persist", bufs=1))
        ACCT = persist.tile([128, DBLK * 132], BF16)

        if STAGE >= 2:
            # ---------- edge phase, NPASS passes of PC chunks ----------
            with ExitStack() as ph:
                sbp = ph.enter_context(tc.tile_pool(name="sbp", bufs=2))
                pse = ph.enter_context(tc.tile_pool(name="pse", bufs=2, space="PSUM"))
                ps2 = ph.enter_context(tc.tile_pool(name="ps2", bufs=2, space="PSUM"))
                psa = ph.enter_context(tc.tile_pool(name="psa", bufs=4, space="PSUM"))

                def _emit_agg(prv):
                    pc0, poh, pSCv = prv
                    for cc in range(PC):
                        cg = pc0 + cc
                        bkt = cg // 2
                        first = (cg % 2 == 0)
                        last = (cg % 2 == 1)
                        if first:
                            pac = psa.tile([128, 132], F32, space="PSUM", tag="pacc")
                            _emit_agg.pac = pac
                        pac = _emit_agg.pac
                        nc.tensor.matmul(pac[:], lhsT=poh[:, cc * 128:(cc + 1) * 128],
                                         rhs=pSCv[:, cc, :], start=first, stop=last)
                        if last:
                            nc.vector.tensor_copy(
                                ACCT[:, bkt * 132:(bkt + 1) * 132], pac[:])
                    blo, bhi = pc0 // 2, (pc0 + PC) // 2
                    nc.sync.dma_start(acc_tbl[:, blo * 132:bhi * 132],
                                      ACCT[:, blo * 132:bhi * 132])

                xg = sbp.tile([128, 1, CAP], BF16, tag="xg", bufs=1)
                # <=896 idxs per transpose-gather (DGE per-inst limit)
                off = 0
                while off < CAP:
                    n = min(896, CAP - off)
                    nc.gpsimd.dma_gather(
                        xg[:, :, off:off + n], x_rows[:],
                        hsI_t[:, off // 16:(off + n) // 16],
                        n, n, 128, transpose=True)
                    off += n
                xgv = xg[:].rearrange("p a b -> p (a b)")
                prev = None
                for p in range(NPASS):
                    c0 = p * PC
                    oh_t = sbp.tile([128, PC * 128], BF16, tag="oh")
                    nc.sync.dma_start(oh_t[:], ohC[:, c0 * 128:(c0 + PC) * 128])
                    ohT_t = sbp.tile([128, PC * 128], BF16, tag="ohT")
                    nc.sync.dma_start(ohT_t[:], ohT[:, c0 * 128:(c0 + PC) * 128])
                    SH = sbp.tile([128, PC * 132], BF16, tag="SH")
                    SHv = SH[:].rearrange("p (e w) -> p e w", w=132)
                    padst = ps2.tile([128, PC * 4], F32, space="PSUM", tag="padst")
                    for cc in range(PC):
                        if cc % 2 == 0:
                            ph_ = pse.tile([128, 264], F32, space="PSUM", tag="ph")
                        nc.tensor.matmul(ph_[:, (cc % 2) * 132:(cc % 2) * 132 + 132],
                                         lhsT=xgv[0:F_IN, (c0 + cc) * 128:(c0 + cc + 1) * 128],
                                         rhs=wbig_t[:], start=True, stop=True)
                        nc.tensor.matmul(padst[:, cc * 4:(cc + 1) * 4],
                                         lhsT=ohT_t[:, cc * 128:(cc + 1) * 128],
                                         rhs=ADTb[:, ((c0 + cc) // 2) * 4:
                                                  ((c0 + cc) // 2) * 4 + 4],
                                         start=True, stop=True)
                        if cc % 2 == 1:
                            nc.scalar.mul(SH[:, (cc - 1) * 132:(cc + 1) * 132],
                                          ph_[:], 1.0)
                    if prev is not None:
                        _emit_agg(prev)
                    # alpha math, batched over the pass
                    ADx = sbp.tile([128, PC * 4], BF16, tag="ADx", bufs=1)
                    nc.vector.tensor_copy(ADx[:], padst[:])
                    ADxv = ADx[:].rearrange("p (e w) -> p e w", w=4)
                    EAm = sbp.tile([128, PC], F32, tag="EAm", bufs=1)
                    nc.vector.scalar_tensor_tensor(
                        out=EAm[:], in0=sm_t[:, c0:c0 + PC], scalar=meanr[:],
                        op0=ALU.mult, op1=ALU.add, in1=ea_t[:, c0:c0 + PC])
                    Q = sbp.tile([128, PC * 4], F32, tag="Q", bufs=1)
                    Qv = Q[:].rearrange("p (e w) -> p e w", w=4)
                    nc.vector.tensor_tensor(out=Qv, in0=SHv[:, :, 128:132],
                                            in1=ADxv, op=ALU.add)
                    T2 = sbp.tile([128, PC * 4], F32, tag="T2", bufs=1)
                    T2v = T2[:].rearrange("p (e w) -> p e w", w=4)
                    ea3 = EAm[:].rearrange("p (e w) -> p e w", w=1) \
                        .to_broadcast([128, PC, 4])
                    kap3 = kap_t[:].rearrange("p (o w) -> p o w", o=1) \
                        .to_broadcast([128, PC, 4])
                    nc.vector.tensor_tensor(out=T2v, in0=ea3, in1=kap3, op=ALU.mult)
                    nc.vector.tensor_tensor(out=Qv, in0=Qv, in1=T2v, op=ALU.add)
                    nc.vector.tensor_scalar_mul(T2v, Qv, LEAK)
                    nc.vector.tensor_tensor(out=Qv, in0=Qv, in1=T2v, op=ALU.max)
                    SG1 = sbp.tile([128, PC * 4], F32, tag="SG1", bufs=1)
                    nc.scalar.activation(SG1[:], Q[:], AF.Sigmoid)
                    S4 = sbp.tile([128, PC * 4], F32, tag="S4", bufs=1)
                    nc.scalar.activation(S4[:], Q[:], AF.Sigmoid, scale=-1.0)
                    nc.vector.reciprocal(S4[:], S4[:])
                    nc.vector.tensor_tensor(out=S4[:], in0=SG1[:], in1=S4[:],
                                            op=ALU.mult)
                    S4b = sbp.tile([128, PC * 4], BF16, tag="S4b", bufs=1)
                    nc.vector.tensor_copy(S4b[:], S4[:])
                    S4bv = S4b[:].rearrange("p (e w) -> p e w", w=4)
                    SCT = sbp.tile([128, PC * 132], BF16, tag="SCT")
                    SCv = SCT[:].rearrange("p (e w) -> p e w", w=132)
                    nc.vector.tensor_copy(SCv[:, :, 128:132], S4bv)
                    nh4 = SHv[:, :, 0:128].rearrange("p e (h c) -> p e h c", h=HEADS)
                    sc4 = S4bv.rearrange("p e (h c) -> p e h c", c=1) \
                        .to_broadcast([128, PC, HEADS, C])
                    out4 = SCv[:, :, 0:128].rearrange("p e (h c) -> p e h c", h=HEADS)
                    nc.vector.tensor_tensor(out=out4, in0=nh4, in1=sc4, op=ALU.mult)
                    prev = (c0, oh_t, SCv)
                _emit_agg(prev)
            if debug:
                ACF = persist.tile([128, DBLK * 132], F32)
                nc.vector.tensor_copy(ACF[:], ACCT[:])
                nc.sync.dma_start(dbg_acct[:], ACF[:])

        if STAGE >= 3:
            # ---------- ReduceScatter (bf16) ----------
            nc.gpsimd.collective_compute("ReduceScatter", ALU.add, replica_groups=RG,
                                         ins=[acc_tbl[:]], outs=[rs_out[:]])

            # ---------- normalize + ReLU on 128 partitions ----------
            with ExitStack() as ph:
                sbn = ph.enter_context(tc.tile_pool(name="sbn", bufs=1))
                RSb = sbn.tile([128, 10 * 132], BF16)
                nc.sync.dma_start(RSb[:],
                                  rs_out[:].rearrange("p (a w) -> (p a) w", a=8))
                RSv = RSb[:].rearrange("p (j w) -> p j w", w=132)
                D4 = sbn.tile([128, 40], F32)
                D4v = D4[:].rearrange("p (j w) -> p j w", w=4)
                nc.vector.tensor_scalar_add(D4v, RSv[:, :, 128:132], 1e-16)
                nc.vector.reciprocal(D4[:], D4[:])
                RC = sbn.tile([128, 40], BF16)
                nc.vector.tensor_copy(RC[:], D4[:])
                RCv = RC[:].rearrange("p (j w) -> p j w", w=4)
                GBn = sbn.tile([128, 10 * 128], BF16)
                g4 = GBn[:].rearrange("p (j h c) -> p j h c", j=10, h=HEADS)
                m4 = RSv[:, :, 0:128].rearrange("p j (h c) -> p j h c", h=HEADS)
                r4 = RCv.rearrange("p j (h c) -> p j h c", c=1) \
                    .to_broadcast([128, 10, HEADS, C])
                nc.vector.tensor_tensor(out=g4, in0=m4, in1=r4, op=ALU.mult)
                GBv = GBn[:].rearrange("p (j w) -> p j w", w=128)
                gbb = gb_t[:].rearrange("p (o w) -> p o w", o=1) \
                    .to_broadcast([128, 10, 128])
                nc.vector.tensor_tensor(out=GBv, in0=GBv, in1=gbb, op=ALU.add)
                nc.vector.tensor_scalar_max(GBn[:], GBn[:], 0.0)
                # node-major writes: node t = r*80 + a*10 + bb  (partition = r*8+a)
                nc.sync.dma_start(
                    gat_loc[:].rearrange("(ra bb) w -> ra (bb w)", bb=10), GBn[:])
                nc.sync.dma_start(
                    ghalo_in[:].rearrange("(ra bb) w -> ra (bb w)", bb=10),
                    GBn[112:128, :])
                if debug:
                    GBF = sbn.tile([128, 10 * 128], F32)
                    nc.vector.tensor_copy(GBF[:], GBn[:])
                    nc.sync.dma_start(dbg_gbn[:], GBF[:])

        if STAGE >= 4:
            # ---------- halo AllGather + slice assembly ----------
            nc.gpsimd.collective_compute("AllGather", ALU.bypass, replica_groups=RG,
                                         ins=[ghalo_in[:]], outs=[ghalo_full[:]])

        lstm = top.enter_context(tc.tile_pool(name="lstm", bufs=1))
        GT = lstm.tile([128, SEQ], BF16)
        gx = lstm.tile([128, SEQ], F32)
        H = lstm.tile([HID, SEQ + 1], F32)
        nc.vector.memset(H[:, 0:1], 0.0)
        if STAGE < 5:
            nc.vector.memset(gx[:], 0.0)
            nc.vector.memset(GT[:], 0.0)
            nc.vector.memset(H[:], 0.0)
        if STAGE >= 5:
            with ExitStack() as ph:
                sbg = ph.enter_context(tc.tile_pool(name="sbg", bufs=1))
                psg = ph.enter_context(tc.tile_pool(name="psg", bufs=2, space="PSUM"))
                GTH = sbg.tile([128, 1, 256], BF16)
                nc.gpsimd.dma_gather(GTH[:], ghalo_full[:], slH_t[:],
                                     256, 256, 128, transpose=True)
                nc.vector.tensor_copy(GT[:, 0:HALO],
                                      GTH[:].rearrange("p a b -> p (a b)")[:, 0:HALO])
                nc.sync.dma_start_transpose(GT[:, HALO:SEQ], gat_loc[:])
                for lo, w in ((0, 512), (512, 512), (1024, 416)):
                    pg = psg.tile([128, 512], F32, space="PSUM", tag="pg")
                    nc.tensor.matmul(pg[:, 0:w], lhsT=wih_t[:], rhs=GT[:, lo:lo + w],
                                     start=True, stop=True)
                    nc.vector.tensor_scalar_add(gx[:, lo:lo + w], pg[:, 0:w], br_t[:])
                # halo kill (core 0): gx = gx*hm + hk
                nc.vector.scalar_tensor_tensor(
                    out=gx[:, 0:HALO], in0=gx[:, 0:HALO], scalar=hm_t[:],
                    op0=ALU.mult, op1=ALU.add,
                    in1=hk_t[:].to_broadcast([128, HALO]))
        if debug:
            dgt = lstm.tile([128, SEQ], F32)
            nc.vector.tensor_copy(dgt[:], GT[:])
            nc.sync.dma_start(dbg_gt[:], dgt[:])
            nc.sync.dma_start(dbg_gx[:], gx[:])

        if STAGE >= 6:
            # ---------- LSTM fixed point on [*, SEQ] ----------
            with ExitStack() as ph:
                sbl = ph.enter_context(tc.tile_pool(name="sbl", bufs=2))
                psl = ph.enter_context(tc.tile_pool(name="psl", bufs=1, space="PSUM"))
                use_f32r = os.environ.get("KF32R", "0") == "1"
                whh_r = whh_t[:].bitcast(dt.float32r) if use_f32r else whh_t[:]
                for it in range(ITERS):
                    if it == 0:
                        Gp = gx[:]
                    else:
                        pG = psl.tile([128, SEQ], F32, space="PSUM", tag="pG")
                        for lo, w in ((0, 512), (512, 512), (1024, 416)):
                            rh = H[:, lo:lo + w]
                            if use_f32r:
                                rh = rh.bitcast(dt.float32r)
                            nc.tensor.matmul(pG[:, lo:lo + w], lhsT=whh_r,
                                             rhs=rh, start=True, stop=True)
                        Gs = sbl.tile([128, SEQ], F32, tag="Gs")
                        nc.vector.tensor_tensor(out=Gs[:], in0=pG[:], in1=gx[:],
                                                op=ALU.add)
                        Gp = Gs[:]
                    S_ = sbl.tile([96, SEQ], F32, tag="S")
                    nc.scalar.activation(S_[:], Gp[0:96, :], AF.Sigmoid)
                    Tg = sbl.tile([64, SEQ], F32, tag="Tg")
                    nc.scalar.activation(Tg[32:64, :], Gp[96:128, :], AF.Tanh)
                    Zt = sbl.tile([HID, SEQ], F32, tag="Zt")
                    nc.vector.tensor_tensor(out=Zt[:], in0=S_[32:64, :],
                                            in1=Tg[32:64, :], op=ALU.mult)
                    Ct = sbl.tile([HID, SEQ], F32, tag="Ct")
                    nc.vector.tensor_tensor_scan(
                        out=Ct[:], data0=S_[0:32, :], data1=Zt[:],
                        initial=0.0, op0=ALU.mult, op1=ALU.add)
                    TC = sbl.tile([96, SEQ], F32, tag="TC")
                    nc.scalar.activation(TC[64:96, :], Ct[:], AF.Tanh)
                    nc.vector.tensor_tensor(out=H[:, 1:SEQ + 1], in0=S_[64:96, :],
                                            in1=TC[64:96, :], op=ALU.mult)
        if debug:
            nc.sync.dma_start(dbg_h[:], H[:, 1:SEQ + 1])

        if STAGE >= 7:
            # ---------- FC + final AllGather ----------
            with ExitStack() as ph:
                sbf = ph.enter_context(tc.tile_pool(name="sbf", bufs=1))
                psf = ph.enter_context(tc.tile_pool(name="psf", bufs=2, space="PSUM"))
                OF = sbf.tile([1, DSL], F32)
                for lo, w in ((0, 512), (512, 512), (1024, 256)):
                    pf = psf.tile([1, 512], F32, space="PSUM", tag="pf")
                    nc.tensor.matmul(pf[:, 0:w], lhsT=wfc_t[:],
                                     rhs=H[:, HALO + 1 + lo:HALO + 1 + lo + w],
                                     start=True, stop=True)
                    nc.vector.tensor_scalar_add(OF[:, lo:lo + w], pf[:, 0:w], bfc_t[:])
                nc.sync.dma_start(fc_out[:], OF[:])
                nc.gpsimd.collective_compute("AllGather", ALU.bypass,
                                             replica_groups=RG,
                                             ins=[fc_out[:]], outs=[fc_full[:]])
                OT = sbf.tile([NCORES, DSL], F32)
                nc.sync.dma_start(OT[:], fc_full[:])
                nc.sync.dma_start(out[:], OT[:])

    nc.compile()
    return nc


def run(inputs, trace=False, debug=False):
    key = ("dbg" if debug else "rel")
    if key not in _CACHE:
        _CACHE[key] = _build_nc(debug=debug)
    nc = _CACHE[key]
    in_maps = _prep_host(inputs)
    res = run_bass_kernel_spmd(nc, in_maps, list(range(NCORES)), trace=trace)
    return res


def kernel(**inputs) -> np.ndarray:
    res = run(inputs)
    o = res.results[0]["out"]
    return np.ascontiguousarray(
        o.reshape(NCORES * DSL)[:N].reshape(N, 1).astype(np.float32))
